# revision 1
# baseline (speedup 1.0000x reference)
"""Trainium2 Bass kernel for nn_COLoss_45457933860953.

Loss = mean over all pixels of weighted -log(conf gathered by instance)
     + mean over batches of (masked offset MSE sum / fg count).

Data-parallel over the batch dim: 16 batches -> 8 cores x 2 batches.
The instance map (values 0/1) is shipped as int8 (lossless) to cut DMA
bytes ~10%; C=2 turns the gather into a predicated copy; both loss
reductions use fused multiply+free-sum (scalar_tensor_tensor accum).

Each core emits [128, 6] per-partition partials:
  col 0: sum log(g)          (both batches)
  col 1: sum m*log(g)        (both batches)
  col 2: sum m*((g0-o0)^2 + (g1-o1)^2) batch 0
  col 3: same, batch 1
  col 4: count(m) batch 0
  col 5: count(m) batch 1
Host combines in float64:
  conf_loss = -(0.4*S1 + 0.6*S2)/N        (weight = 0.4 + 0.6*m)
  off_loss  = mean_b(sums_b / counts_b)
"""

import sys

if "/opt/trn_rl_repo" not in sys.path:
    sys.path.insert(0, "/opt/trn_rl_repo")

import numpy as np

import concourse.bass as bass
import concourse.tile as tile
from concourse import mybir
from concourse.bass_utils import run_bass_kernel_spmd

B, C, H, W = 16, 2, 512, 512
NCORES = 8
BPC = B // NCORES            # batches per core
P = 128                      # SBUF partitions
FREE = (H * W) // P          # 2048 free elems per partition per image
CHUNKS = (1024, 1024)        # 4KB per partition per DMA = full DMA BW
NCHUNK = len(CHUNKS)
NSETS = BPC * NCHUNK         # chunk-sets per core

F32 = mybir.dt.float32
I8 = mybir.dt.int8
AF = mybir.ActivationFunctionType
ALU = mybir.AluOpType


def _legalize_single_wait(nc):
    """This toolchain's walrus accepts at most ONE sync-wait on TPB compute
    instructions and rejects the EVENT_SEMAPHORE_RANGE_CLEAR InstISA that
    TileContext emits in its kernel tail. Drop the range clear (sems are
    not recycled in a one-shot NEFF) and hoist surplus waits onto
    standalone single-wait InstEventSemaphore carriers placed immediately
    before the instruction on the same engine queue (prefix waits on an
    in-order queue are semantically identical to instruction waits)."""
    cnt = 0
    for f in nc.m.functions:
        for blk in f.blocks:
            out = []
            for ins in blk.instructions:
                nm = type(ins).__name__
                if (nm == "InstISA" and
                        getattr(ins, "op_name", None) ==
                        "EVENT_SEMAPHORE_RANGE_CLEAR"):
                    continue
                si = getattr(ins, "sync_info", None)
                if si is not None and si.on_wait and len(si.on_wait) > 1:
                    waits = list(si.on_wait)
                    for w in waits[:-1]:
                        cnt += 1
                        out.append(mybir.InstEventSemaphore(
                            name=f"{ins.name}-hoist{cnt}",
                            engine=ins.engine,
                            ins=[], outs=[],
                            sync_info=mybir.SyncInfo(on_wait=[w],
                                                     on_update=[]),
                        ))
                    ins.sync_info = mybir.SyncInfo(
                        on_wait=[waits[-1]], on_update=list(si.on_update))
                out.append(ins)
            blk.instructions = out
    return nc


def build_nc(legalize=True):
    nc = bass.Bass("TRN2", target_bir_lowering=False, debug=False,
                   num_devices=NCORES)
    # ~2 MB DMAs sustain ~380 GB/s (1 MB: ~335, 3 MB single-stream: ~326).
    # conf: one 2 MB DMA per batch; off+gto packed on host: 2 MB per chunk.
    conf = nc.dram_tensor("conf", [BPC, C, H, W], F32, kind="ExternalInput")
    bgo = nc.dram_tensor("bgo", [BPC, 2 * C, H, W], F32,
                         kind="ExternalInput")
    inst = nc.dram_tensor("inst", [BPC, 1, H, W], I8, kind="ExternalInput")
    out = nc.dram_tensor("partials", [P, 6], F32, kind="ExternalOutput")

    # [b, c, (p q), w] -> [p, b, c, (q w)]: partition p holds 4 contiguous
    # image rows; any column slice is contiguous per partition.
    conf_r = conf.rearrange("b c (p q) w -> p b c (q w)", p=P)
    bgo_r = bgo.rearrange("b c (p q) w -> p b c (q w)", p=P)
    inst_r = inst.rearrange("b c (p q) w -> p b (c q w)", p=P)
    OF0, GT0 = 0, 2                      # channel indices in `bgo`

    def acc_tiles(pool, base, n):
        return [pool.tile([P, 1], F32, name=f"{base}{i}", tag=f"{base}{i}")
                for i in range(n)]

    with tile.TileContext(nc) as tc:
        with (
            tc.tile_pool(name="io", bufs=4) as io,
            tc.tile_pool(name="work", bufs=4) as work,
            tc.tile_pool(name="acc", bufs=1) as accp,
        ):
            # per-partial accumulator tiles (a single shared strip tile
            # measured ~5us slower: cross-engine shared-tile WAW waits
            # serialize the queues)
            NL = NSETS + 1   # conf partials (+1: last chunk in halves)
            NO = NSETS * C + C  # offset partials (+C: last chunk halved)
            lg_s = acc_tiles(accp, "lg_s", NL)     # sum log(g)
            mlg_s = acc_tiles(accp, "mlg_s", NL)   # sum m*log(g)
            cnt_s = acc_tiles(accp, "cnt_s", BPC)  # count(m) per batch
            off_s = acc_tiles(accp, "off_s", NO)   # masked offset sq sums
            zb = accp.tile([P, 1], F32)            # zero bias for ACT

            nc.vector.memset(zb[:], 0.0)
            res = accp.tile([P, 6], F32)
            lgsub = acc_tiles(accp, "lgsub", BPC)
            mlgsub = acc_tiles(accp, "mlgsub", BPC)

            for bi in range(BPC):
                # full-batch mask: one DMA, count once on ACT (off the
                # critical path), chunk slices feed the masked reductions
                mask_t = io.tile([P, FREE], I8, name="mask_t", tag="mask_t",
                                 bufs=2)
                nc.sync.dma_start(mask_t[:], inst_r[:, bi, :])
                instf = work.tile([P, FREE], F32, name="instf", tag="instf",
                                  bufs=2)
                nc.scalar.activation(instf[:], mask_t[:], AF.Copy,
                                     accum_out=cnt_s[bi][:])
                nc.vector.tensor_copy(res[:, 4 + bi:5 + bi], cnt_s[bi][:])

                # whole-batch conf (2 MB); lands while bgo chunks stream,
                # so the conf chain never sits on the kernel tail
                conf_t = io.tile([P, C, FREE], F32, name="conf_t",
                                 tag="conf_t", bufs=2)
                nc.sync.dma_start(conf_t[:], conf_r[:, bi, :, :])

                bgo_ts = []
                col = 0
                for j, T in enumerate(CHUNKS):
                    b_t = io.tile([P, 2 * C, CHUNKS[0]], F32, name="b_t",
                                  tag="b_t")
                    nc.sync.dma_start(b_t[:, :, :T],
                                      bgo_r[:, bi, :, col:col + T])
                    bgo_ts.append(b_t)
                    col += T

                for j, T in enumerate(CHUNKS):
                    si = bi * NCHUNK + j
                    last = (bi == BPC - 1 and j == NCHUNK - 1)
                    halves = 2 if last else 1
                    hs = T // halves
                    base = sum(CHUNKS[:j])

                    # conf path: CP -> Ln(+free-sum) -> masked free-sum
                    for h in range(halves):
                        lcol = si if h == 0 else NSETS
                        hsl = slice(base + h * hs, base + (h + 1) * hs)
                        g = conf_t[:, 0, hsl]
                        mh = mask_t[:, hsl]
                        nc.vector.copy_predicated(g, mh, conf_t[:, 1, hsl])
                        nc.scalar.activation(g, g, AF.Ln, bias=zb[:],
                                             accum_out=lg_s[lcol][:])
                        nc.vector.scalar_tensor_tensor(
                            out=g, in0=g, scalar=1.0, in1=mh,
                            op0=ALU.mult, op1=ALU.mult,
                            accum_out=mlg_s[lcol][:])

                    # offset path: sub -> square -> masked free-sum;
                    # c0 subs ride the idle Pool engine except on the
                    # kernel tail, where Pool is ~3x slower than DVE
                    b_t = bgo_ts[j]
                    for c in range(C):
                        for h in range(halves):
                            ocol = (si * C + c) if h == 0 else (NSETS * C + c)
                            tsl = slice(h * hs, (h + 1) * hs)
                            mh = mask_t[:, base + h * hs:base + (h + 1) * hs]
                            d = work.tile([P, CHUNKS[0]], F32, name=f"d{c}",
                                          tag=f"d{c}")
                            dv = d[:, :hs]
                            eng = nc.vector if (last or c == 1) \
                                else nc.gpsimd
                            eng.tensor_sub(dv, b_t[:, GT0 + c, tsl],
                                           b_t[:, OF0 + c, tsl])
                            nc.scalar.activation(dv, dv, AF.Square,
                                                 bias=zb[:])
                            nc.vector.scalar_tensor_tensor(
                                out=dv, in0=dv, scalar=1.0, in1=mh,
                                op0=ALU.mult, op1=ALU.mult,
                                accum_out=off_s[ocol][:])

                    # fold this chunk's partials into running subtotals so
                    # only ~6 tiny adds remain on the kernel tail
                    ro = res[:, 2 + bi:3 + bi]
                    cols = [si * C, si * C + 1]
                    if last:
                        cols += [NSETS * C, NSETS * C + 1]
                    if j == 0:
                        nc.vector.tensor_add(ro, off_s[cols[0]][:],
                                             off_s[cols[1]][:])
                        cols = cols[2:]
                    for oc in cols:
                        nc.vector.tensor_add(ro, ro, off_s[oc][:])
                    if j == NCHUNK - 1:
                        nc.vector.tensor_add(lgsub[bi][:], lg_s[si - 1][:],
                                             lg_s[si][:])
                        nc.vector.tensor_add(mlgsub[bi][:],
                                             mlg_s[si - 1][:], mlg_s[si][:])
                        if last:
                            nc.vector.tensor_add(lgsub[bi][:], lgsub[bi][:],
                                                 lg_s[NSETS][:])
                            nc.vector.tensor_add(mlgsub[bi][:],
                                                 mlgsub[bi][:],
                                                 mlg_s[NSETS][:])

            nc.vector.tensor_add(res[:, 0:1], lgsub[0][:], lgsub[1][:])
            nc.vector.tensor_add(res[:, 1:2], mlgsub[0][:], mlgsub[1][:])
            nc.sync.dma_start(out[:, :], res[:])

    return _legalize_single_wait(nc) if legalize else nc


_NC = None


def _get_nc():
    global _NC
    if _NC is None:
        _NC = build_nc()
    return _NC


def make_in_maps(confidence, offset, instance, gt_offset):
    confidence = np.ascontiguousarray(confidence, dtype=np.float32)
    offset = np.ascontiguousarray(offset, dtype=np.float32)
    gt_offset = np.ascontiguousarray(gt_offset, dtype=np.float32)
    bgo = np.concatenate([offset, gt_offset], axis=1)
    # values are 0/1: int8 is lossless and cuts DMA bytes ~10%
    inst8 = np.asarray(instance).astype(np.int8)
    in_maps = []
    for k in range(NCORES):
        sl = slice(BPC * k, BPC * (k + 1))
        in_maps.append({"conf": confidence[sl], "bgo": bgo[sl],
                        "inst": inst8[sl]})
    return in_maps


def combine_partials(parts):
    """parts: list of 8 arrays [128, 6] -> scalar loss (float64)."""
    s1 = sum(p[:, 0].sum(dtype=np.float64) for p in parts)
    s2 = sum(p[:, 1].sum(dtype=np.float64) for p in parts)
    n = float(B * H * W)
    conf_loss = -(0.4 * s1 + 0.6 * s2) / n
    off_loss = 0.0
    for p in parts:
        for bi in range(BPC):
            s = p[:, 2 + bi].sum(dtype=np.float64)
            cnt = p[:, 4 + bi].sum(dtype=np.float64)
            if cnt > 0.5:
                off_loss += s / cnt
    off_loss /= B
    return conf_loss + off_loss


def kernel(confidence, offset, instance, gt_offset):
    nc = _get_nc()
    in_maps = make_in_maps(confidence, offset, instance, gt_offset)
    res = run_bass_kernel_spmd(nc, in_maps, core_ids=list(range(NCORES)))
    parts = [r["partials"] for r in res.results]
    return np.array(combine_partials(parts), dtype=np.float32)



# revision 9
# speedup vs baseline: 1.2514x; 1.2514x over previous
"""Trainium2 Bass kernel for nn_COLoss_45457933860953.

Loss = mean over all pixels of weighted -log(conf gathered by instance)
     + mean over batches of (masked offset MSE sum / fg count).

Data-parallel over the batch dim: 16 batches -> 8 cores x 2 batches.

Inputs are compressed to bf16 on the host (loss tolerance is 2e-2; bf16
quantization contributes ~5e-6 rel err, measured) and packed per core
into per-chunk contiguous blobs so each chunk is ONE fully-contiguous
2D DMA (128 rows x 14/7 KB):

  chunk layout per partition (ck = 1024 or 512 cols):
    [ mask | conf0 | conf1 | off0 | off1 | gt0 | gt1 ]   (7*ck bf16)

Partition p holds image rows 4p..4p+3 flattened (2048 cols per batch);
each batch is split into col-chunks [1024, 512, 512] (small tail chunk
keeps the post-DMA critical path short).

Per chunk s, engines:
  PE  : count = ones[128,1]^T @ mask strips -> psum[1,512] (per batch)
  DVE : gather g=conf[inst] (copy_predicated), d_c = gt_c - off_c,
        dm_c = d_c * m, S2 accum (stt m*log g)
  ACT : log g (accum S1), Square(dm_c) (accum off sums)

Outputs: racc[128,18] (S1 per chunk, off sq-sums per chunk*ch),
vacc[128,6] (S2 per chunk), cnt0/cnt1[1,512] psum counts.
Host combines in float64 (identical formula to the fp32 baseline).
"""

import sys

if "/opt/trn_rl_repo" not in sys.path:
    sys.path.insert(0, "/opt/trn_rl_repo")

import ml_dtypes
import numpy as np

import concourse.bass as bass
import concourse.tile as tile
from concourse import mybir
from concourse.bass_utils import run_bass_kernel_spmd

B, C, H, W = 16, 2, 512, 512
NCORES = 8
BPC = B // NCORES            # batches per core
P = 128                      # SBUF partitions
FREE = (H * W) // P          # 2048 cols per partition per image
CHUNKS = (1024, 512, 512)    # col-chunks per batch; small tail chunk
NCHUNK = len(CHUNKS)
NSETS = BPC * NCHUNK         # chunk-sets per core (6)
NFIELD = 7                   # mask, c0, c1, o0, o1, g0, g1

BF16 = mybir.dt.bfloat16
F32 = mybir.dt.float32
AF = mybir.ActivationFunctionType
ALU = mybir.AluOpType
NPBF16 = ml_dtypes.bfloat16


def _legalize_single_wait(nc):
    """This toolchain's walrus accepts at most ONE sync-wait on TPB compute
    instructions and rejects the EVENT_SEMAPHORE_RANGE_CLEAR InstISA that
    TileContext emits in its kernel tail. Drop the range clear (sems are
    not recycled in a one-shot NEFF) and hoist surplus waits onto
    standalone single-wait InstEventSemaphore carriers placed immediately
    before the instruction on the same engine queue (prefix waits on an
    in-order queue are semantically identical to instruction waits)."""
    cnt = 0
    for f in nc.m.functions:
        for blk in f.blocks:
            out = []
            for ins in blk.instructions:
                nm = type(ins).__name__
                if (nm == "InstISA" and
                        getattr(ins, "op_name", None) ==
                        "EVENT_SEMAPHORE_RANGE_CLEAR"):
                    continue
                si = getattr(ins, "sync_info", None)
                if si is not None and si.on_wait and len(si.on_wait) > 1:
                    waits = list(si.on_wait)
                    for w in waits[:-1]:
                        cnt += 1
                        out.append(mybir.InstEventSemaphore(
                            name=f"{ins.name}-hoist{cnt}",
                            engine=ins.engine,
                            ins=[], outs=[],
                            sync_info=mybir.SyncInfo(on_wait=[w],
                                                     on_update=[]),
                        ))
                    ins.sync_info = mybir.SyncInfo(
                        on_wait=[waits[-1]], on_update=list(si.on_update))
                out.append(ins)
            blk.instructions = out
    return nc


def build_nc(legalize=True):
    nc = bass.Bass("TRN2", target_bir_lowering=False, debug=False,
                   num_devices=NCORES)
    # one contiguous 2D DMA per chunk-set: [128 rows x NFIELD*ck bf16]
    blobA = nc.dram_tensor("blobA", [BPC, P, NFIELD * CHUNKS[0]], BF16,
                           kind="ExternalInput")
    blobB = nc.dram_tensor("blobB", [BPC * 2, P, NFIELD * CHUNKS[1]], BF16,
                           kind="ExternalInput")
    racc_d = nc.dram_tensor("racc", [P, 3 * NSETS], F32,
                            kind="ExternalOutput")
    vacc_d = nc.dram_tensor("vacc", [P, NSETS], F32, kind="ExternalOutput")
    cnt_d = nc.dram_tensor("cnt", [1, BPC * 512], F32, kind="ExternalOutput")

    with tile.TileContext(nc) as tc:
        with (
            tc.tile_pool(name="io", bufs=1) as io,
            tc.tile_pool(name="work", bufs=3) as work,
            tc.tile_pool(name="acc", bufs=1) as accp,
            tc.tile_pool(name="ps", bufs=1, space="PSUM") as psp,
        ):
            # ---- issue ALL input DMAs up front (sync queue) -------------
            # 6 chunk tiles live simultaneously (~70 KB/partition total);
            # the 16 DMA rings then stream back-to-back with no buffer
            # stalls, and compute chases chunk completion.
            tiles = []
            bidx = [0, 0]        # next index into blobA / blobB
            for bi in range(BPC):
                for c, ck in enumerate(CHUNKS):
                    s = bi * NCHUNK + c
                    if ck == CHUNKS[0]:
                        T = io.tile([P, NFIELD * ck], BF16, name=f"tA{s}",
                                    tag=f"tA{s}")
                        nc.sync.dma_start(T[:], blobA[bidx[0]])
                        bidx[0] += 1
                    else:
                        T = io.tile([P, NFIELD * ck], BF16, name=f"tB{s}",
                                    tag=f"tB{s}")
                        nc.sync.dma_start(T[:], blobB[bidx[1]])
                        bidx[1] += 1
                    tiles.append((T, ck))

            # ---- constants + ACT table prefetch -------------------------
            ones = accp.tile([P, 1], BF16)
            nc.gpsimd.memset(ones[:], 1.0)
            dummy = accp.tile([P, 1], BF16)
            # first Ln triggers the ~2.7us ACT_TABLE_LOAD; issue it at t=0
            # so it overlaps the first chunk DMA instead of serializing
            nc.scalar.activation(dummy[:], ones[:], AF.Ln)

            racc = accp.tile([P, 3 * NSETS], F32)
            vacc = accp.tile([P, NSETS], F32)
            cnt_ps = [psp.tile([1, 512], F32, name=f"cnt{b}", tag=f"cnt{b}")
                      for b in range(BPC)]
            cnt_sb = accp.tile([1, BPC * 512], F32)

            # ---- per-chunk compute --------------------------------------
            for bi in range(BPC):
                strips_total = sum(ck // 512 for ck in CHUNKS)
                strip_i = 0
                for c, ck in enumerate(CHUNKS):
                    s = bi * NCHUNK + c
                    T, _ = tiles[s]
                    m = T[:, 0:ck]
                    c0 = T[:, ck:2 * ck]
                    c1 = T[:, 2 * ck:3 * ck]
                    o0 = T[:, 3 * ck:4 * ck]
                    o1 = T[:, 4 * ck:5 * ck]
                    g0 = T[:, 5 * ck:6 * ck]
                    g1 = T[:, 6 * ck:7 * ck]

                    # fg count via idle PE: psum[1,512] += ones^T @ m
                    for st in range(ck // 512):
                        nc.tensor.matmul(
                            cnt_ps[bi][:, :], ones[:],
                            m[:, st * 512:(st + 1) * 512],
                            start=(strip_i == 0),
                            stop=(strip_i == strips_total - 1))
                        strip_i += 1

                    # conf: gather into c0, then Ln -> c1 (accum S1).
                    # copy_predicated needs an int8 predicate: view the
                    # bf16 mask's high byte (1.0 -> 0x3F, 0.0 -> 0x00)
                    m_i8 = m.bitcast(mybir.dt.int8).rearrange(
                        "p (k two) -> p k two", two=2)[:, :, 1]
                    nc.vector.copy_predicated(c0, m_i8, c1)
                    if c == NCHUNK - 1:
                        # counts done for this batch; PSUM isn't DMA-able,
                        # bounce through SBUF (b0's copy hides mid-stream)
                        nc.vector.tensor_copy(
                            cnt_sb[0:1, 512 * bi:512 * (bi + 1)],
                            cnt_ps[bi][:])
                    # offsets: d = gt - o, then dm = d*m (Square on ACT)
                    d0 = work.tile([P, CHUNKS[0]], BF16, name="d0", tag="d0")
                    d1 = work.tile([P, CHUNKS[0]], BF16, name="d1", tag="d1")
                    nc.vector.tensor_sub(d0[:, :ck], g0, o0)
                    nc.vector.tensor_mul(d0[:, :ck], d0[:, :ck], m)
                    nc.vector.tensor_sub(d1[:, :ck], g1, o1)
                    nc.vector.tensor_mul(d1[:, :ck], d1[:, :ck], m)

                    nc.scalar.activation(c1, c0, AF.Ln,
                                         accum_out=racc[:, s:s + 1])
                    nc.scalar.activation(
                        d0[:, :ck], d0[:, :ck], AF.Square,
                        accum_out=racc[:, NSETS + 2 * s:NSETS + 2 * s + 1])
                    nc.scalar.activation(
                        d1[:, :ck], d1[:, :ck], AF.Square,
                        accum_out=racc[:, NSETS + 2 * s + 1:
                                       NSETS + 2 * s + 2])

                    # S2 = sum m*log g (DVE last: Ln is long done by now)
                    nc.vector.scalar_tensor_tensor(
                        out=c0, in0=c1, scalar=1.0, in1=m,
                        op0=ALU.mult, op1=ALU.mult,
                        accum_out=vacc[:, s:s + 1])

            # ---- results out -------------------------------------------
            nc.sync.dma_start(racc_d[:, :], racc[:])
            nc.gpsimd.dma_start(vacc_d[:, :], vacc[:])
            nc.sync.dma_start(cnt_d[:, :], cnt_sb[:])

    return _legalize_single_wait(nc) if legalize else nc


_NC = None


def _get_nc():
    global _NC
    if _NC is None:
        _NC = build_nc()
    return _NC


def make_in_maps(confidence, offset, instance, gt_offset):
    conf = np.ascontiguousarray(confidence, dtype=np.float32) \
        .reshape(B, C, P, FREE).astype(NPBF16)
    off = np.ascontiguousarray(offset, dtype=np.float32) \
        .reshape(B, 2, P, FREE).astype(NPBF16)
    gto = np.ascontiguousarray(gt_offset, dtype=np.float32) \
        .reshape(B, 2, P, FREE).astype(NPBF16)
    mask = (np.asarray(instance).reshape(B, P, FREE) != 0).astype(NPBF16)

    def pack(b, lo, hi):
        return np.concatenate(
            [mask[b][:, lo:hi], conf[b, 0][:, lo:hi], conf[b, 1][:, lo:hi],
             off[b, 0][:, lo:hi], off[b, 1][:, lo:hi],
             gto[b, 0][:, lo:hi], gto[b, 1][:, lo:hi]], axis=1)

    edges = np.cumsum((0,) + CHUNKS)
    in_maps = []
    for k in range(NCORES):
        bs = [BPC * k + i for i in range(BPC)]
        blobA = np.stack([pack(b, edges[0], edges[1]) for b in bs])
        blobB = np.stack([pack(b, edges[c], edges[c + 1])
                          for b in bs for c in (1, 2)])
        in_maps.append({"blobA": blobA, "blobB": blobB})
    return in_maps


def combine_partials(parts):
    """parts: list of 8 dicts (racc [P,18], vacc [P,6], cnt [2,512])
    -> scalar loss (float64)."""
    s1 = sum(p["racc"][:, 0:NSETS].sum(dtype=np.float64) for p in parts)
    s2 = sum(p["vacc"].sum(dtype=np.float64) for p in parts)
    n = float(B * H * W)
    conf_loss = -(0.4 * s1 + 0.6 * s2) / n
    off_loss = 0.0
    for p in parts:
        for bi in range(BPC):
            lo = NSETS + 2 * NCHUNK * bi
            s = p["racc"][:, lo:lo + 2 * NCHUNK].sum(dtype=np.float64)
            cntb = p["cnt"].reshape(BPC, 512)[bi].sum(dtype=np.float64)
            if cntb > 0.5:
                off_loss += s / cntb
    off_loss /= B
    return conf_loss + off_loss


def kernel(confidence, offset, instance, gt_offset):
    nc = _get_nc()
    in_maps = make_in_maps(confidence, offset, instance, gt_offset)
    res = run_bass_kernel_spmd(nc, in_maps, core_ids=list(range(NCORES)))
    parts = [{k: np.asarray(r[k], dtype=np.float64)
              for k in ("racc", "vacc", "cnt")} for r in res.results]
    return np.array(combine_partials(parts), dtype=np.float32)
